# revision 16
# baseline (speedup 1.0000x reference)
"""Trainium2 Bass kernel for nn_EquiSchNet (gnn_message_passing).

Strategy (data-parallel over batch, 2 samples per core on 8 cores):
  - Residual trunk h kept feature-major in SBUF: hT [128H, 2*512 atoms].
  - Per layer:
      xf = h @ conv_lin1  -> atom-major bf16 gather table in DRAM [512, 256]
      edge MLP (colored): MM1 feature-major (stationary W1_c), softplus on ACT,
        MM2 with the softplus tile as the *stationary* operand -> per-edge
        weight W in edge-major layout straight out of PE (no transpose).
      gather xf rows with the production GPSIMD dma_gather (bf16, 512B rows),
      multiply by W on DVE, scatter-add via block one-hot matmuls into PSUM.
      The cosine cutoff is folded into the one-hot values host-side.
      agg -> PE transpose -> lin2 -> softplus (shift folded into next bias)
        -> blk_lin -> residual add.
  - Readout algebra folded host-side: y = rowsum(hT) . (out1_W@out2_W) + const.

Edges are sorted host-side by (dst_block, color) and padded per bucket to a
multiple of 128 so every 128-edge tile has a single dst block and color.
"""

import math
import numpy as np

_F16 = np.float16

BS, A1, A2 = 16, 256, 256
AT = A1 + A2
E, H, F, L, NG, NC = 24576, 128, 128, 6, 50, 4
CUTOFF = 10.0
SHIFT = float(np.log(2.0))
P = 128
NCORES = 8
BPC = BS // NCORES  # samples per core
DBLK = AT // P  # dst blocks
GTILES = 8  # max tiles per dma_gather piece (1024 idxs; >=2048 overflows DMA rings)
CCH = 4  # tiles per compute chunk


def _ssp(x):
    # shifted softplus: log(1+e^x) - log(2), computed as the device does
    return np.logaddexp(0.0, np.asarray(x, np.float64)).astype(np.float32) - np.float32(
        SHIFT
    )


def _host_edge_plan(edge_idx, edge_weight, colors):
    """Sort edges by (dst_block, color), pad buckets to 128 multiples, and
    build all edge-structure-derived device arrays."""
    src = np.asarray(edge_idx)[:, 0].astype(np.int64)
    dst = np.asarray(edge_idx)[:, 1].astype(np.int64)
    col = np.asarray(colors).astype(np.int64)
    w = np.asarray(edge_weight).astype(np.float32)

    offset = np.linspace(0.0, CUTOFF, NG).astype(np.float32)
    coeff = -0.5 / float(offset[1] - offset[0]) ** 2
    eattr = np.exp(coeff * (w[:, None] - offset[None, :]) ** 2).astype(np.float32)
    ccut = (0.5 * (np.cos(w * np.pi / CUTOFF) + 1.0)).astype(np.float32)

    key = (dst // P) * NC + col
    src_p, dst_p, cc_p, ea_p = [], [], [], []
    buckets = []  # (db, c, tile0, ntiles)
    t0 = 0
    for k in range(DBLK * NC):
        sel = np.where(key == k)[0]
        n = len(sel)
        if n == 0:
            continue
        nt = (n + P - 1) // P
        padn = nt * P - n
        src_p.append(src[sel])
        src_p.append(np.zeros(padn, np.int64))
        dst_p.append(dst[sel])
        dst_p.append(np.full(padn, (k // NC) * P, np.int64))
        cc_p.append(ccut[sel])
        cc_p.append(np.zeros(padn, np.float32))
        ea_p.append(eattr[sel])
        ea_p.append(np.zeros((padn, NG), np.float32))
        buckets.append((k // NC, k % NC, t0, nt))
        t0 += nt
    T = t0
    Epad = T * P
    src_p = np.concatenate(src_p)
    dst_p = np.concatenate(dst_p)
    cc_p = np.concatenate(cc_p)
    ea_p = np.concatenate(ea_p, axis=0)

    # scatter one-hots with cutoff folded in: S[p, t*128 + m]
    S = np.zeros((P, T * P), np.float32)
    n = np.arange(Epad)
    t = n // P
    p = n % P
    tdb = np.zeros(T, np.int64)
    for db, c, bt0, bnt in buckets:
        tdb[bt0 : bt0 + bnt] = db
    m = dst_p - tdb[t] * P
    assert ((m >= 0) & (m < P)).all()
    S[p, t * P + m] = cc_p

    # dma_gather indices, int16, wrapped in 16 partitions, replicated to 128
    C16 = Epad // 16
    blk = np.zeros((16, C16), np.int16)
    blk[n % 16, n // 16] = src_p.astype(np.int16)
    idx16 = np.tile(blk, (8, 1))

    eaT = np.ascontiguousarray(ea_p.T)  # [NG, Epad]

    return dict(
        T=T,
        Epad=Epad,
        buckets=buckets,
        S=S,
        idx16=idx16,
        eaT=eaT,
        src_p=src_p,
        dst_p=dst_p,
        cc_p=cc_p,
        ea_p=ea_p,
        tdb=tdb,
    )


def _host_weights(inp):
    """Weight/bias arrays in device layouts (numpy float32; cast later)."""
    W1 = np.zeros((NG, L * NC * F), np.float32)
    W2 = np.zeros((F, L * NC * F), np.float32)
    BE = np.zeros((F, L * NC), np.float32)  # per (l,c) bias vec (along F)
    B1 = np.zeros((F, L * NC), np.float32)
    for l in range(L):
        for c in range(NC):
            o = (l * NC + c) * F
            W1[:, o : o + F] = inp["mlp_W1"][l, c]
            W2[:, o : o + F] = inp["mlp_W2"][l, c]
            BE[:, l * NC + c] = inp["mlp_b2"][l, c]
            B1[:, l * NC + c] = inp["mlp_b1"][l, c]
    # BE materialized as [128, L*NC*F] bf16 tiles with identical rows
    BEt = np.zeros((P, L * NC * F), np.float32)
    for i in range(L * NC):
        BEt[:, i * F : (i + 1) * F] = BE[:, i][None, :]
    LIN1 = np.concatenate([inp["conv_lin1_W"][l] for l in range(L)], axis=1)  # [H, L*F]
    LIN2 = np.concatenate([inp["conv_lin2_W"][l] for l in range(L)], axis=1)  # [F, L*H]
    BLK = np.concatenate([inp["blk_lin_W"][l] for l in range(L)], axis=1)  # [H, L*H]
    L2B = np.stack([inp["conv_lin2_b"][l] for l in range(L)], axis=1)  # [H, L]
    BKB = np.stack([inp["blk_lin_b"][l] for l in range(L)], axis=1)  # [H, L]
    use_be = bool(np.abs(BE).max() > 0)
    V = (inp["out1_W"] @ inp["out2_W"]).astype(np.float32)  # [H, 1]
    rconst = float(AT * (inp["out1_b"] @ inp["out2_W"] + inp["out2_b"])[0])
    return dict(
        W1=W1,
        W2=W2,
        BEt=BEt,
        B1=B1,
        LIN1=LIN1,
        LIN2=LIN2,
        BLK=BLK,
        L2B=L2B,
        BKB=BKB,
        E1W=np.asarray(inp["emb1_W"], np.float32),
        E2W=np.asarray(inp["emb2_W"], np.float32),
        E1B=np.asarray(inp["emb1_b"], np.float32).reshape(H, 1),
        E2B=np.asarray(inp["emb2_b"], np.float32).reshape(H, 1),
        V=V,
        rconst=rconst,
        use_be=use_be,
    )


def _chunks_of_bucket(bt0, bnt):
    """Split a bucket's tiles into gather pieces (<=GTILES tiles) and compute
    chunks (<=CCH tiles) within each piece."""
    pieces = []
    t = bt0
    while t < bt0 + bnt:
        pn = min(GTILES, bt0 + bnt - t)
        chunks = []
        u = t
        while u < t + pn:
            cn = min(CCH, t + pn - u)
            chunks.append((u, cn))
            u += cn
        pieces.append((t, pn, chunks))
        t += pn
    return pieces


def _bf16(x):
    return np.asarray(x, dtype=_F16)


def _emulate_core(plan, wts, sitesA, sitesP, use_bf16=True):
    """Pure-numpy emulation of the device dataflow for one core (BPC samples).

    sitesA: [1, BPC*A1]; sitesP: [2, BPC*A2]. Returns y [BPC, 1] (without
    rconst).
    """

    def rd(x):  # round through bf16 where the device uses bf16
        return _bf16(x).astype(np.float32) if use_bf16 else np.asarray(x, np.float32)

    T = plan["T"]
    S = rd(plan["S"])
    eaT = rd(plan["eaT"])
    src_p = plan["src_p"]
    W1 = rd(wts["W1"])
    W2 = rd(wts["W2"])
    BEt = np.asarray(wts["BEt"], np.float32)

    # embeddings, feature-major hT [H, BPC*AT]
    hT = np.zeros((H, BPC * AT), np.float32)
    for s in range(BPC):
        h1 = wts["E1W"].T @ sitesA[:, s * A1 : (s + 1) * A1] + wts["E1B"]
        h2 = wts["E2W"].T @ sitesP[:, s * A2 : (s + 1) * A2] + wts["E2B"]
        hT[:, s * AT : s * AT + A1] = h1
        hT[:, s * AT + A1 : (s + 1) * AT] = h2

    for l in range(L):
        lin1 = wts["LIN1"][:, l * F : (l + 1) * F]
        # xf table [AT, BPC*F] bf16
        table = np.zeros((AT, BPC * F), np.float32)
        for s in range(BPC):
            for b in range(DBLK):
                blk = hT[:, s * AT + b * P : s * AT + (b + 1) * P]  # [H, 128a]
                xf = blk.T @ lin1  # [128a, F] fp32 psum
                table[b * P : (b + 1) * P, s * F : (s + 1) * F] = rd(xf)
        agg = np.zeros((DBLK, P, BPC * F), np.float32)
        for db, c, bt0, bnt in plan["buckets"]:
            o = (l * NC + c) * F
            w1 = W1[:, o : o + F]
            w2 = W2[:, o : o + F]
            be = BEt[:, o : o + F]
            b1 = wts["B1"][:, l * NC + c : l * NC + c + 1]
            for pt0, pn, chunks in _chunks_of_bucket(bt0, bnt):
                gath = rd(table[src_p[pt0 * P : (pt0 + pn) * P]])  # [pn*128, 256]
                for u, cn in chunks:
                    cols = slice(u * P, (u + cn) * P)
                    t1 = eaT[:, cols].T @ w1 + b1.T  # [cn*128, F]
                    t1s = rd(_ssp(t1))
                    for i in range(cn):
                        tl = u + i
                        wpsum = t1s[i * P : (i + 1) * P] @ w2  # [128, F] fp32
                        if wts["use_be"]:
                            wpsum = rd(wpsum + be)
                        g = gath[(tl - pt0) * P : (tl - pt0 + 1) * P]  # [128, 256]
                        msg = rd(g.reshape(P, BPC, F) * wpsum[:, None, :]).reshape(
                            P, BPC * F
                        )
                        agg[db] += S[:, tl * P : (tl + 1) * P].T @ msg
        # dense tail
        lin2 = wts["LIN2"][:, l * H : (l + 1) * H]
        blkw = wts["BLK"][:, l * H : (l + 1) * H]
        l2b = wts["L2B"][:, l]
        bkb = wts["BKB"][:, l]
        hT_new = hT.copy()
        for s in range(BPC):
            aggT = np.zeros((F, AT), np.float32)
            for b in range(DBLK):
                aggT[:, b * P : (b + 1) * P] = agg[b][:, s * F : (s + 1) * F].T
            x2 = lin2.T @ aggT  # [H, AT]
            soft2 = _ssp(x2 + l2b[:, None])
            x3 = blkw.T @ soft2
            hT_new[:, s * AT : (s + 1) * AT] = (
                hT[:, s * AT : (s + 1) * AT] + x3 + bkb[:, None]
            )
        hT = hT_new

    y = np.zeros((BPC, 1), np.float32)
    for s in range(BPC):
        hsum = hT[:, s * AT : (s + 1) * AT].sum(axis=1)
        y[s, 0] = hsum @ wts["V"][:, 0]
    return y


# ---------------------------------------------------------------------------
# Bass program
# ---------------------------------------------------------------------------

_PROGRAM_CACHE = {}


def _build_program(T, buckets, use_be=False):
    import concourse.bass as bass
    import concourse.tile as tile
    import concourse.mybir as mybir
    from concourse import bacc
    from concourse.masks import make_identity
    from contextlib import ExitStack

    dt = mybir.dt
    Epad = T * P

    nc = bacc.Bacc("TRN2", target_bir_lowering=False, debug=False, num_devices=NCORES)

    def xin(name, shape, d):
        return nc.dram_tensor(name, shape, d, kind="ExternalInput").ap()

    S_in = xin("S", [P, T * P], dt.float16)
    ea_in = xin("eaT", [NG, Epad], dt.float16)
    idx_in = xin("idx16", [P, Epad // 16], dt.int16)
    w1_in = xin("W1", [NG, L * NC * F], dt.float16)
    w2_in = xin("W2", [F, L * NC * F], dt.float16)
    be_in = xin("BEt", [P, L * NC * F], dt.float32)
    b1_in = xin("B1", [F, L * NC], dt.float32)
    lin1_in = xin("LIN1", [H, L * F], dt.float32)
    lin2_in = xin("LIN2", [F, L * H], dt.float32)
    blk_in = xin("BLK", [H, L * H], dt.float32)
    l2b_in = xin("L2B", [H, L], dt.float32)
    bkb_in = xin("BKB", [H, L], dt.float32)
    e1w_in = xin("E1W", [1, H], dt.float32)
    e2w_in = xin("E2W", [2, H], dt.float32)
    e1b_in = xin("E1B", [H, 1], dt.float32)
    e2b_in = xin("E2B", [H, 1], dt.float32)
    v_in = xin("V", [H, 1], dt.float32)
    sa_in = xin("sitesA", [1, BPC * A1], dt.float32)
    sp_in = xin("sitesP", [2, BPC * A2], dt.float32)
    y_out = nc.dram_tensor("y", [BPC, 1], dt.float32, kind="ExternalOutput").ap()
    tables = [
        nc.dram_tensor(f"table{i}", [AT, BPC * F], dt.float16).ap() for i in range(2)
    ]

    with tile.TileContext(nc) as tc, ExitStack() as ctx:
        const = ctx.enter_context(tc.tile_pool(name="const", bufs=1))
        work = ctx.enter_context(tc.tile_pool(name="work", bufs=1))
        ps = ctx.enter_context(tc.tile_pool(name="ps", bufs=1, space="PSUM"))

        _cnt = [0]

        def cload(ap_in, shape, d, engine=None):
            _cnt[0] += 1
            nm = f"c{_cnt[0]}_{ap_in.tensor.name}"
            t = const.tile(shape, d, tag=nm, name=nm)
            (engine or nc.sync).dma_start(t[:], ap_in[:])
            return t

        S_sb = cload(S_in, [P, T * P], dt.float16)
        ea_sb = cload(ea_in, [NG, Epad], dt.float16)
        idx_sb = cload(idx_in, [P, Epad // 16], dt.int16)
        w1_sb = cload(w1_in, [NG, L * NC * F], dt.float16)
        w2_sb = cload(w2_in, [F, L * NC * F], dt.float16)
        be_sb = cload(be_in, [P, L * NC * F], dt.float32)
        b1_sb = cload(b1_in, [F, L * NC], dt.float32)
        lin1_sb = cload(lin1_in, [H, L * F], dt.float32)
        lin2_sb = cload(lin2_in, [F, L * H], dt.float32)
        blk_sb = cload(blk_in, [H, L * H], dt.float32)
        l2b_sb = cload(l2b_in, [H, L], dt.float32)
        bkb_sb = cload(bkb_in, [H, L], dt.float32)
        e1w_sb = cload(e1w_in, [1, H], dt.float32)
        e2w_sb = cload(e2w_in, [2, H], dt.float32)
        e1b_sb = cload(e1b_in, [H, 1], dt.float32)
        e2b_sb = cload(e2b_in, [H, 1], dt.float32)
        v_sb = cload(v_in, [H, 1], dt.float32)
        sa_sb = cload(sa_in, [1, BPC * A1], dt.float32)
        sp_sb = cload(sp_in, [2, BPC * A2], dt.float32)
        identity = const.tile([P, P], dt.float32, tag="identity", name="identity")
        make_identity(nc, identity[:])
        halfc = const.tile([P, 1], dt.float32, tag="halfc", name="halfc")
        nc.vector.memset(halfc[:], 0.5)

        Ident = mybir.ActivationFunctionType.Identity
        ExpF = mybir.ActivationFunctionType.Exp
        LnF = mybir.ActivationFunctionType.Ln
        MUL = mybir.AluOpType.mult
        ADD = mybir.AluOpType.add

        def ssp(out_ap, in_ap, tmp_ap, bias):
            # out = log(1 + exp(in + bias)) - log(2) == log(.5*exp(in+bias) + .5)
            nc.scalar.activation(tmp_ap, in_ap, ExpF, bias=bias)
            nc.scalar.activation(out_ap, tmp_ap, LnF, bias=halfc[:, 0:1], scale=0.5)

        _pcnt = [0]

        def psum(shape, tag, bufs):
            _pcnt[0] += 1
            return ps.tile(
                shape, dt.float32, tag=tag, bufs=bufs, name=f"ps_{tag}_{_pcnt[0]}"
            )

        # ----- embeddings -> hT
        h0p = psum([P, BPC * AT], "agg", 1)
        for s in range(BPC):
            nc.tensor.matmul(
                h0p[:, s * AT : s * AT + A1],
                lhsT=e1w_sb[:1, :],
                rhs=sa_sb[:1, s * A1 : (s + 1) * A1],
                start=True,
                stop=True,
            )
            nc.tensor.matmul(
                h0p[:, s * AT + A1 : (s + 1) * AT],
                lhsT=e2w_sb[:2, :],
                rhs=sp_sb[:2, s * A2 : (s + 1) * A2],
                start=True,
                stop=True,
            )
        hT = work.tile([P, BPC * AT], dt.float32, tag="hT", bufs=2)
        for s in range(BPC):
            nc.scalar.activation(
                hT[:, s * AT : s * AT + A1],
                h0p[:, s * AT : s * AT + A1],
                Ident,
                bias=e1b_sb[:, 0:1],
            )
            nc.scalar.activation(
                hT[:, s * AT + A1 : (s + 1) * AT],
                h0p[:, s * AT + A1 : (s + 1) * AT],
                Ident,
                bias=e2b_sb[:, 0:1],
            )

        for l in range(L):
            table = tables[l % 2]
            # ----- xf = h @ lin1 -> bf16 gather table (atom-major)
            xfsb = work.tile([P, BPC * AT], dt.float16, tag="xf", bufs=2)
            for half in range(2):  # two psum tiles of 2 blocks each
                xfp = psum([P, 512], "mm", 3)
                for q in range(2):
                    b = half * 2 + q
                    for s in range(BPC):
                        nc.tensor.matmul(
                            xfp[:, q * 256 + s * F : q * 256 + (s + 1) * F],
                            lhsT=hT[:, s * AT + b * P : s * AT + (b + 1) * P],
                            rhs=lin1_sb[:, l * F : (l + 1) * F],
                            start=True,
                            stop=True,
                        )
                nc.scalar.activation(
                    xfsb[:, half * 512 : (half + 1) * 512], xfp[:], Ident
                )
            for b in range(DBLK):
                nc.sync.dma_start(
                    table[b * P : (b + 1) * P, :],
                    xfsb[:, b * 256 : (b + 1) * 256],
                )

            # ----- edge pipeline
            aggp = psum([P, BPC * AT], "agg", 1)
            first_db = [True] * DBLK
            ntile_db = [0] * DBLK
            for db, c, bt0, bnt in buckets:
                ntile_db[db] += bnt
            done_db = [0] * DBLK
            for db, c, bt0, bnt in buckets:
                o = (l * NC + c) * F
                for pt0, pn, chunks in _chunks_of_bucket(bt0, bnt):
                    gath = work.tile(
                        [P, GTILES * BPC * F], dt.float16, tag="gath", bufs=3
                    )
                    nc.gpsimd.dma_gather(
                        gath[:, : pn * BPC * F].rearrange(
                            "p (t f) -> p t f", f=BPC * F
                        ),
                        table[:],
                        idx_sb[:, pt0 * 8 : (pt0 + pn) * 8],
                        pn * P,
                        pn * P,
                        BPC * F,
                    )
                    for u, cn in chunks:
                        nn = cn * P
                        t1p = psum([P, 512], "mm", 3)
                        nc.tensor.matmul(
                            t1p[:, :nn],
                            lhsT=w1_sb[:, o : o + F],
                            rhs=ea_sb[:, u * P : u * P + nn],
                            start=True,
                            stop=True,
                        )
                        t1s = work.tile([P, 512], dt.float16, tag="t1s", bufs=2)
                        t1e = work.tile([P, 512], dt.float32, tag="t1e", bufs=2)
                        ssp(
                            t1s[:, :nn],
                            t1p[:, :nn],
                            t1e[:, :nn],
                            b1_sb[:, l * NC + c : l * NC + c + 1],
                        )
                        wp = psum([P, 512], "w", 2)
                        for i in range(cn):
                            nc.tensor.matmul(
                                wp[:, i * F : (i + 1) * F],
                                lhsT=t1s[:, i * F : (i + 1) * F],
                                rhs=w2_sb[:, o : o + F],
                                start=True,
                                stop=True,
                            )
                        msg = work.tile(
                            [P, CCH * BPC * F], dt.float16, tag="msg", bufs=3
                        )
                        goff = (u - pt0) * BPC * F
                        if use_be:
                            wb = work.tile([P, 512], dt.float16, tag="wb", bufs=2)
                            nc.vector.tensor_tensor(
                                wb[:, :nn].rearrange("p (t f) -> p t f", f=F),
                                wp[:, :nn].rearrange("p (t f) -> p t f", f=F),
                                be_sb[:, o : o + F]
                                .rearrange("p (u f) -> p u f", u=1)
                                .to_broadcast([P, cn, F]),
                                ADD,
                            )
                            w_src = wb[:, :nn]
                        else:
                            w_src = wp[:, :nn]
                        nc.vector.tensor_tensor(
                            msg[:, : cn * BPC * F].rearrange(
                                "p (t s f) -> p t s f", s=BPC, f=F
                            ),
                            gath[:, goff : goff + cn * BPC * F].rearrange(
                                "p (t s f) -> p t s f", s=BPC, f=F
                            ),
                            w_src.rearrange("p (t u f) -> p t u f", u=1, f=F)
                            .to_broadcast([P, cn, BPC, F]),
                            MUL,
                        )
                        for i in range(cn):
                            tl = u + i
                            done_db[db] += 1
                            nc.tensor.matmul(
                                aggp[:, db * BPC * F : (db + 1) * BPC * F],
                                lhsT=S_sb[:, tl * P : (tl + 1) * P],
                                rhs=msg[:, i * BPC * F : (i + 1) * BPC * F],
                                start=first_db[db],
                                stop=done_db[db] == ntile_db[db],
                            )
                            first_db[db] = False

            # ----- dense tail
            aggsb = work.tile([P, BPC * AT], dt.float32, tag="aggsb", bufs=1)
            for db in range(DBLK):
                if ntile_db[db] == 0:
                    nc.vector.memset(
                        aggp[:, db * BPC * F : (db + 1) * BPC * F], 0.0
                    )
                for s in range(BPC):
                    nc.scalar.activation(
                        aggsb[:, (db * BPC + s) * F : (db * BPC + s + 1) * F],
                        aggp[:, db * BPC * F + s * F : db * BPC * F + (s + 1) * F],
                        Ident,
                    )
            aggT = work.tile([P, BPC * AT], dt.float32, tag="aggT", bufs=1)
            for s in range(BPC):
                trp = psum([P, 512], "tail", 1)
                for b in range(DBLK):
                    nc.tensor.transpose(
                        trp[:, b * P : (b + 1) * P],
                        aggsb[:, (b * BPC + s) * F : (b * BPC + s + 1) * F],
                        identity[:],
                    )
                nc.scalar.activation(aggT[:, s * AT : (s + 1) * AT], trp[:], Ident)
            soft2 = work.tile([P, BPC * AT], dt.float32, tag="soft2", bufs=1)
            for s in range(BPC):
                x2p = psum([P, 512], "tail", 1)
                nc.tensor.matmul(
                    x2p[:],
                    lhsT=lin2_sb[:, l * H : (l + 1) * H],
                    rhs=aggT[:, s * AT : (s + 1) * AT],
                    start=True,
                    stop=True,
                )
                x2e = work.tile([P, 512], dt.float32, tag="t1e", bufs=2)
                ssp(
                    soft2[:, s * AT : (s + 1) * AT],
                    x2p[:],
                    x2e[:],
                    l2b_sb[:, l : l + 1],
                )
            hT_new = work.tile([P, BPC * AT], dt.float32, tag="hT", bufs=2)
            for s in range(BPC):
                x3p = psum([P, 512], "tail", 1)
                nc.tensor.matmul(
                    x3p[:],
                    lhsT=blk_sb[:, l * H : (l + 1) * H],
                    rhs=soft2[:, s * AT : (s + 1) * AT],
                    start=True,
                    stop=True,
                )
                nc.vector.scalar_tensor_tensor(
                    hT_new[:, s * AT : (s + 1) * AT],
                    x3p[:],
                    bkb_sb[:, l : l + 1],
                    hT[:, s * AT : (s + 1) * AT],
                    ADD,
                    ADD,
                )
            hT = hT_new

        # ----- readout
        hsum = work.tile([P, BPC], dt.float32, tag="hsum", bufs=1)
        import concourse.mybir as _mb

        for s in range(BPC):
            nc.vector.reduce_sum(
                hsum[:, s : s + 1],
                hT[:, s * AT : (s + 1) * AT],
                _mb.AxisListType.X,
            )
        rop = psum([P, 512], "tail", 1)
        nc.tensor.matmul(
            rop[:BPC, :1], lhsT=hsum[:], rhs=v_sb[:], start=True, stop=True
        )
        ysb = work.tile([BPC, 1], dt.float32, tag="y", bufs=1)
        nc.scalar.activation(ysb[:], rop[:BPC, :1], Ident)
        nc.sync.dma_start(y_out[:], ysb[:])

    nc.compile()
    return nc


def _prep(inputs):
    plan = _host_edge_plan(inputs["edge_idx"], inputs["edge_weight"], inputs["colors"])
    wts = _host_weights(inputs)
    shared = {
        "S": _bf16(plan["S"]),
        "eaT": _bf16(plan["eaT"]),
        "idx16": plan["idx16"],
        "W1": _bf16(wts["W1"]),
        "W2": _bf16(wts["W2"]),
        "BEt": np.asarray(wts["BEt"], np.float32),
        "B1": wts["B1"],
        "LIN1": wts["LIN1"],
        "LIN2": wts["LIN2"],
        "BLK": wts["BLK"],
        "L2B": wts["L2B"],
        "BKB": wts["BKB"],
        "E1W": wts["E1W"],
        "E2W": wts["E2W"],
        "E1B": wts["E1B"],
        "E2B": wts["E2B"],
        "V": wts["V"],
    }
    sites = np.asarray(inputs["sites"], np.float32)
    sites_p = np.asarray(inputs["sites_p"], np.float32)
    in_maps = []
    for core in range(NCORES):
        m = dict(shared)
        sA = np.zeros((1, BPC * A1), np.float32)
        sP = np.zeros((2, BPC * A2), np.float32)
        for s in range(BPC):
            b = core * BPC + s
            sA[0, s * A1 : (s + 1) * A1] = sites[b, :, 0]
            sP[:, s * A2 : (s + 1) * A2] = sites_p[b].T
        m["sitesA"] = sA
        m["sitesP"] = sP
        in_maps.append(m)
    return plan, wts, in_maps


def kernel(**inputs) -> np.ndarray:
    from concourse.bass_utils import run_bass_kernel_spmd

    plan, wts, in_maps = _prep(inputs)
    key = (plan["T"], tuple(plan["buckets"]), wts["use_be"])
    if key not in _PROGRAM_CACHE:
        _PROGRAM_CACHE[key] = _build_program(
            plan["T"], plan["buckets"], use_be=wts["use_be"]
        )
    nc = _PROGRAM_CACHE[key]
    res = run_bass_kernel_spmd(nc, in_maps, list(range(NCORES)))
    out = np.zeros((BS, 1), np.float32)
    for core in range(NCORES):
        out[core * BPC : (core + 1) * BPC] = res.results[core]["y"] + wts["rconst"]
    return out
